# revision 30
# baseline (speedup 1.0000x reference)
"""Trainium2 Bass kernel for nn_Cross_attention2 (dense transformer cross-attention).

Math (per batch b, head h), faithful to the reference module:
    Q = q @ W_h + b_h ; K = k @ W_h + b_h ; V = v @ W_h + b_h
    alpha = (Q K^T)/sqrt(512); masked -> -1e9; alpha /= sqrt(512); P = softmax(alpha)
    out[b, :, h*512:(h+1)*512] = P @ V

Device algorithm (algebraically identical post-softmax):
    G    = W_h W_h^T, Wb = W_h b_h     (host-precomputed, fp8-e4m3 x16)
    Z    = G q^T + (16 Wb) 1^T  (fp8, = 16*(G q^T + Wb 1^T))           [512, Lq]
    s^T  = k @ Z = 16*(QK^T)^T minus per-q/constant shifts (softmax-invariant)
    P^T  = exp(s^T/8192) * mask^T   (multiplicative 0/1 mask == additive -inf)
    sums = column sums of P^T (per q);  V = v @ W_h + 1 b_h^T  (bias folded here)
    O    = (P^T)^T @ V, scaled per-partition by 1/sums
Sharding: 2 batch-groups x 4 head-groups on 8 cores (8 batches, 2 heads each).
alpha-path matmuls (Z, s) run fp8-e4m3 DoubleRow (2x PE rate); the softmax's
flatness (alpha ~ N(0, 1/512)) makes their quantization error negligible.
V and O matmuls stay fp16 (their error passes straight through).
"""

import os
import sys
from contextlib import ExitStack

import numpy as np

for _p in ("/opt/trn_rl_repo",):
    if os.path.isdir(_p) and _p not in sys.path:
        sys.path.append(_p)

import concourse.bacc as bacc
import concourse.mybir as mybir
import concourse.tile as tile
from concourse.bass import ts
from concourse.bass_utils import run_bass_kernel_spmd

dt = mybir.dt

B, L, D, H = 16, 512, 512, 8
NCORES = 8
BGROUPS, HGROUPS = 2, 4          # core grid: 2 batch-groups x 4 head-groups
BPC = B // BGROUPS               # 8 batches per core
HPC = H // HGROUPS               # 2 heads per core
C = D // 128  # 128-row chunks per 512
NPAIR = C // 2  # DoubleRow processes chunk pairs
ALPHA = 16.0  # host prescale on What (fp8 subnormal avoidance)
DR = mybir.MatmulPerfMode.DoubleRow

_CACHE = {}


def _build():
    nc = bacc.Bacc("TRN2", target_bir_lowering=False, debug=False, num_devices=NCORES)
    f32 = dt.float32
    f16 = dt.float16
    f8 = dt.float8e4

    # all input tensors are host-packed partition-major [.., 128, C, X] so
    # every DMA moves 2-4KB contiguous per partition line (descriptor
    # efficiency; 512B lines throttle the DMA rings)
    qT_d = nc.dram_tensor("qT", [BPC, 128, C, L], f8, kind="ExternalInput").ap()
    kT_d = nc.dram_tensor("kT", [BPC, 128, C, L], f8, kind="ExternalInput").ap()
    vT_d = nc.dram_tensor("vT", [BPC, 128, C, L], f16, kind="ExternalInput").ap()
    mT_d = nc.dram_tensor("mT", [BPC, 128, C, L], f8, kind="ExternalInput").ap()
    Wn_d = nc.dram_tensor("Wn", [HPC, 128, C, D], f16, kind="ExternalInput").ap()
    G_d = nc.dram_tensor("G8", [HPC, 128, C, D], f8, kind="ExternalInput").ap()
    WbC_d = nc.dram_tensor("WbC", [HPC, 128, C], f32, kind="ExternalInput").ap()
    ones_d = nc.dram_tensor("ones", [128, L], f16, kind="ExternalInput").ap()
    bb_d = nc.dram_tensor("bb", [HPC, 128, D], f32, kind="ExternalInput").ap()
    out_d = nc.dram_tensor("out", [BPC, L, HPC * D], f16, kind="ExternalOutput").ap()

    EXP = mybir.ActivationFunctionType.Exp
    IDENT = mybir.ActivationFunctionType.Identity

    with tile.TileContext(nc) as tc, ExitStack() as ctx:
        const = ctx.enter_context(tc.tile_pool(name="const", bufs=1))
        acts = ctx.enter_context(tc.tile_pool(name="acts", bufs=2))
        headp = ctx.enter_context(tc.tile_pool(name="headp", bufs=1))
        work = ctx.enter_context(tc.tile_pool(name="work", bufs=2))
        psb = ctx.enter_context(tc.tile_pool(name="psb", bufs=4, space="PSUM"))
        pso = ctx.enter_context(tc.tile_pool(name="pso", bufs=2, space="PSUM"))
        pss = ctx.enter_context(tc.tile_pool(name="pss", bufs=1, space="PSUM"))

        # DMA-completion counters are per issuing engine: consumers wait on
        # "all DMAs issued so far on this ring", so spread loads across engine
        # rings by when they're first needed (SP: alpha path; scalar: V path;
        # vector: s/mask path; gpsimd: head-1 weights + output stores).
        def load_q(b, strips=1, eng=None):
            tq = acts.tile([128, C, L], f8, tag="q", name=f"qTs{b}")
            _load_acts_tensor(tq, qT_d[b], strips, eng or nc.sync)
            return tq

        def _load_acts_tensor(t, src, strips, eng):
            # per-chunk-pair DMA strips let the first consumer start early
            if strips <= 1:
                eng.dma_start(t[:], src)
                return
            step = C // strips
            for c0 in range(0, C, step):
                eng.dma_start(t[:, c0 : c0 + step, :], src[:, c0 : c0 + step, :])

        def load_kvm(b, tq, strips=1):
            tk = acts.tile([128, C, L], f8, tag="k", name=f"kTs{b}")
            _load_acts_tensor(tk, kT_d[b], strips, nc.sync)
            tv = acts.tile([128, C, L], f16, tag="v", name=f"vTs{b}")
            _load_acts_tensor(tv, vT_d[b], strips, nc.scalar)
            tm = acts.tile([128, C, L], f8, tag="m", name=f"mTs{b}")
            _load_acts_tensor(tm, mT_d[b], strips, nc.gpsimd)
            return (tq, tk, tv, tm)

        # iteration-0 critical set alone on the SP ring: Ghat h0 + q0,
        # chunk-striped so the first Z matmul starts after ~1/4 lands
        Ghats, WbCols = [], []
        for h in range(HPC):
            g = headp.tile([128, C, D], f8, tag=f"G{h}", name=f"G8s{h}")
            Ghats.append(g)
        # critical first transfers (G8h0 pair0 + q0 pair0) on separate rings so
        # their triggers and transfers overlap; everything else follows
        tq0 = acts.tile([128, C, L], f8, tag="q", name="qTs0")
        nc.sync.dma_start(Ghats[0][:, 0:2, :], G_d[0][:, 0:2, :])
        nc.scalar.dma_start(tq0[:, 0:2, :], qT_d[0][:, 0:2, :])
        nc.gpsimd.dma_start(Ghats[0][:, 2:4, :], G_d[0][:, 2:4, :])
        nc.sync.dma_start(tq0[:, 2:4, :], qT_d[0][:, 2:4, :])
        wbc0 = headp.tile([128, C], f32, tag="WbC0", name="WbCs0")
        nc.sync.dma_start(wbc0[:], WbC_d[0])
        WbCols.append(wbc0)
        onesT = const.tile([128, L], f16, tag="ones")
        nc.sync.dma_start(onesT[:], ones_d[:])
        # V-path weights for h0 on the scalar ring (needed ~2us in)
        Wns, bbs_l = [], []
        w0 = headp.tile([128, C, D], f16, tag="Wn0", name="Wns0")
        nc.scalar.dma_start(w0[:], Wn_d[0])
        Wns.append(w0)
        bb0 = headp.tile([128, D], f32, tag="bb0", name="bbs0")
        nc.scalar.dma_start(bb0[:], bb_d[0])
        bbs_l.append(bb0)
        cur_acts = load_kvm(0, tq0, strips=2)
        # head-1 weights on the gpsimd ring (needed ~12us in)
        nc.gpsimd.dma_start(Ghats[1][:], G_d[1])
        wbc1 = headp.tile([128, C], f32, tag="WbC1", name="WbCs1")
        nc.gpsimd.dma_start(wbc1[:], WbC_d[1])
        WbCols.append(wbc1)
        w1 = headp.tile([128, C, D], f16, tag="Wn1", name="Wns1")
        nc.gpsimd.dma_start(w1[:], Wn_d[1])
        Wns.append(w1)
        bb1 = headp.tile([128, D], f32, tag="bb1", name="bbs1")
        nc.gpsimd.dma_start(bb1[:], bb_d[1])
        bbs_l.append(bb1)

        def emit_sums_O(st, final=False):
            """sums + O for a finished iteration; tiny sums MMs interleaved 1:1
            with big O MMs so the PE activity monitor never sees a lull.
            V carries the output bias (V = vW + 1 b^T), so O needs only the
            1/sums scaling (DVE per-partition scalar mul)."""
            PTsb, Vsb, b, h = st

            def out_tile(u, ops, rsb, ru, split=1):
                Osb = work.tile([128, D], f16, tag="O", bufs=3, name=f"Osb{u}")
                nc.vector.tensor_scalar_mul(Osb[:], ops[:], rsb[:, 2 * ru : 2 * ru + 1])
                w = D // split
                rings = [nc.sync, nc.scalar, nc.gpsimd]
                for j in range(split):
                    eng = rings[(2 * b + h + u + j) % 3]
                    eng.dma_start(
                        out_d[b, ts(u, 128), h * D + j * w : h * D + (j + 1) * w],
                        Osb[:, j * w : (j + 1) * w],
                    )

            # per-pair sums groups: the pair's reciprocal + O-scale run on DVE
            # while the next pair's matmuls still run on the PE, freeing the
            # pair's PSUM "o" slots before the next pair needs them. The final
            # iteration drains at single-u granularity so its output DMA
            # overlaps its remaining matmuls.
            ugroups = (
                [(0,), (1,), (2,), (3,)] if final else [(0, 1), (2, 3)]
            )
            for gi, us in enumerate(ugroups):
                u0 = us[0]
                # alternate sums banks (padded slots) so a group's start=True
                # zero doesn't touch the previous group's still-live bank
                sums = pss.tile(
                    [128, 8], f32, tag=f"sums{(gi % 2) * 2}", name=f"fsums{u0}",
                    padded_shape=[128, 512],
                )
                rsb = work.tile([128, 8], f32, tag="rsb", name=f"frsb{u0}")
                opss = {}
                n = 0
                for t in range(C):
                    for u in us:
                        if t == 0:
                            opss[u] = pso.tile([128, D], f32, tag="o", name=f"fops{u}")
                        nc.tensor.matmul(
                            opss[u][:], PTsb[:, t, ts(u, 128)], Vsb[:, t, :],
                            start=(t == 0), stop=(t == C - 1),
                        )
                        nc.tensor.matmul(
                            sums[:, 2 * (u - u0) : 2 * (u - u0) + 2],
                            PTsb[:, t, ts(u, 128)], onesT[:, 0:2],
                            start=(n == 0), stop=(n == len(us) * C - 1),
                        )
                        n += 1
                nc.vector.reciprocal(rsb[:], sums[:])
                for u in us:
                    out_tile(u, opss[u], rsb, u - u0, split=4 if final else 1)

        pending = None
        for b in range(BPC):
            qTb, kTb, vTb, mTb = cur_acts
            if b + 1 < BPC:
                nxt_acts = load_kvm(b + 1, load_q(b + 1))
            for h in range(HPC):
                Ghat, WbCol = Ghats[h], WbCols[h]

                # Z = G q^T (fp8 DoubleRow); +16Wb bias folded into the
                # psum->sbuf copy as a per-partition ACT bias
                Zsb = work.tile([128, C, L], f8, tag="Z")
                for t in range(C):
                    zps = psb.tile([128, L], f32, tag="big")
                    for cp in range(NPAIR):
                        sl = slice(2 * cp, 2 * cp + 2)
                        nc.tensor.matmul(
                            zps[:], Ghat[:, sl, ts(t, 128)], qTb[:, sl, :],
                            start=(cp == 0), stop=(cp == NPAIR - 1), perf_mode=DR,
                        )
                    nc.scalar.activation(
                        Zsb[:, t, :], zps[:], IDENT,
                        bias=WbCol[:, t : t + 1], scale=1.0,
                    )

                # V = vT^T @ W + 1 b^T (bias folded via the psum->sbuf add)
                Vsb = work.tile([128, C, D], f16, tag="V")
                for t in range(C):
                    vps = psb.tile([128, D], f32, tag="big")
                    for c in range(C):
                        nc.tensor.matmul(
                            vps[:], vTb[:, c, ts(t, 128)], Wns[h][:, c, :],
                            start=(c == 0), stop=(c == C - 1),
                        )
                    nc.vector.tensor_add(Vsb[:, t, :], vps[:], bbs_l[h][:])

                # s^T = kT^T @ Z (fp8 DoubleRow); P^T = exp(s^T/8192) * m^T
                PTsb = work.tile([128, C, L], f16, tag="PT")
                for t in range(C):
                    sps = psb.tile([128, L], f32, tag="big")
                    for cp in range(NPAIR):
                        sl = slice(2 * cp, 2 * cp + 2)
                        nc.tensor.matmul(
                            sps[:], kTb[:, sl, ts(t, 128)], Zsb[:, sl, :],
                            start=(cp == 0), stop=(cp == NPAIR - 1), perf_mode=DR,
                        )
                    pte = work.tile([128, L], f16, tag="sm")
                    nc.scalar.activation(pte[:], sps[:], EXP, scale=1.0 / 8192.0)
                    nc.gpsimd.tensor_mul(PTsb[:, t, :], pte[:], mTb[:, t, :])

                # software pipeline: sums/O of the previous iteration lands
                # here, after this iteration's PE work covered its exp latency
                if pending is not None:
                    emit_sums_O(pending)
                pending = (PTsb, Vsb, b, h)
            if b + 1 < BPC:
                cur_acts = nxt_acts
        emit_sums_O(pending, final=True)

    nc.compile()
    return nc


def _prep_inputs(query, key, value, mask, Wq, bq):
    f = np.float32
    h16 = np.float16

    def pack(xT):  # [N, D_or_L, L] -> partition-major [N, 128, C, L]
        n, dd, ll = xT.shape
        return np.ascontiguousarray(
            xT.reshape(n, dd // 128, 128, ll).transpose(0, 2, 1, 3)
        )

    f8np = dt.np(dt.float8e4)
    qT = pack(np.asarray(query, f).transpose(0, 2, 1).astype(f8np))
    kT = pack(np.asarray(key, f).transpose(0, 2, 1).astype(f8np))
    vT = pack(np.asarray(value, f).transpose(0, 2, 1).astype(h16))
    mT = pack(
        (np.asarray(mask) != 0).astype(f8np).transpose(0, 2, 1)
    )  # [B, Lk->p/c, Lq] multiplicative 0/1 (exact in fp8)
    Wn32 = np.asarray(Wq, f)
    Wn = pack(Wn32.astype(h16))
    # host-precomputed alpha-path weights, mirroring the on-device fp8 math:
    # What = [W; b^T] x16 quantized to e4m3, G8 = fp8(What What^T / 16)
    What = np.concatenate([Wn32, np.asarray(bq, f)[:, None, :]], axis=1)  # [H,513,512]
    Wa8 = (What * ALPHA).astype(f8np).astype(f)  # [H, 513, 512] dequantized
    Gfull = np.einsum("hid,hjd->hij", Wa8, Wa8)  # [H, 513, 513]
    G8 = pack((Gfull[:, :512, :512] / ALPHA).astype(f8np))  # 16*G in fp8
    Wb16 = (Gfull[:, 512, :512] / ALPHA).astype(f)  # 16*Wb  [H, 512]
    WbC = np.ascontiguousarray(
        Wb16.reshape(H, C, 128).transpose(0, 2, 1)
    )  # [H, 128, C]: WbC[h, p, t] = 16*Wb[t*128+p]
    ones = np.ones((128, L), h16)
    bb = np.broadcast_to(np.asarray(bq, f)[:, None, :], (H, 128, D)).copy()

    in_maps = []
    for c in range(NCORES):
        gb, gh = divmod(c, HGROUPS)
        bs = slice(gb * BPC, (gb + 1) * BPC)
        hs = slice(gh * HPC, (gh + 1) * HPC)
        in_maps.append(
            {
                "qT": qT[bs], "kT": kT[bs], "vT": vT[bs], "mT": mT[bs],
                "Wn": np.ascontiguousarray(Wn[hs]),
                "G8": np.ascontiguousarray(G8[hs]),
                "WbC": np.ascontiguousarray(WbC[hs]),
                "ones": ones,
                "bb": np.ascontiguousarray(bb[hs]),
            }
        )
    return in_maps


def _run(inputs, trace=False):
    if "nc" not in _CACHE:
        _CACHE["nc"] = _build()
    nc = _CACHE["nc"]
    in_maps = _prep_inputs(**inputs)
    last_err = None
    for _attempt in range(3):
        try:
            res = run_bass_kernel_spmd(
                nc, in_maps, core_ids=list(range(NCORES)), trace=trace
            )
            break
        except Exception as e:  # transient NRT device errors happen; retry
            last_err = e
    else:
        raise last_err
    out = np.empty((B, L, H * D), np.float32)
    for c in range(NCORES):
        gb, gh = divmod(c, HGROUPS)
        out[gb * BPC : (gb + 1) * BPC, :, gh * HPC * D : (gh + 1) * HPC * D] = (
            res.results[c]["out"].astype(np.float32)
        )
    return out, res


def kernel(**inputs) -> np.ndarray:
    out, _ = _run(inputs, trace=False)
    return out
